# revision 13
# baseline (speedup 1.0000x reference)
"""Multi-head self-attention (B=2, S=2048, E=1024, H=16, D=64) on 8 TRN2 cores.

Sharding: tensor-parallel over (batch, head-group): core c handles batch c//4
and heads [4*(c%4), 4*(c%4)+4). Each core computes its heads' attention output
projected through its slice of Wo; the host sums the 4 partial outputs per
batch and adds the constant bias row (bv @ Wo + bo).

Device-side math (per core, transposed formulation so no transposes needed):
  QT = Wq_c^T @ x^T + bq_c        [256, S]   (bias bk dropped: softmax-invariant)
  KT = Wk_c^T @ x^T               [256, S]
  V  = x @ Wv_c                   [S, 256]   (bias bv folded into host bias row)
  S^T tile = K @ Q^T              (PE, per 128-k-token x 1024-q tile)
  P^T = exp(S^T / 8)              (ACT, no max subtraction: scores ~ N(0,1))
  O^T aug = [V | 1]^T @ P^T       (PE, accumulated over k tiles; row 64 = sum)
  O^T = O^T aug[0:64] / row 64    (recip + PE broadcast + DVE mul)
  Y = O @ Wo_c                    [S, 1024]  fp32 partial out
"""

import numpy as np
import ml_dtypes

import concourse.bass as bass
import concourse.bacc as bacc
import concourse.tile as tile
from concourse import mybir
from concourse.bass_utils import run_bass_kernel_spmd

B, S, E = 2, 2048, 1024
H, D = 16, 64
NCORES = 8
HPC = 4                 # heads per core
EH = HPC * D            # 256: per-core head width
P = 128
EC = E // P             # 8 E-chunks of 128
MC = EH // P            # 2 Eh-chunks of 128
NT = S // P             # 16 token tiles of 128
QH = 1024               # q-chunk processed per attention unit
NQH = S // QH           # 2
SCALE = 1.0 / float(np.sqrt(D))

DT = mybir.dt.bfloat16
NP_DT = ml_dtypes.bfloat16
F32 = mybir.dt.float32
F32R = mybir.dt.float32r

AF = mybir.ActivationFunctionType


def build_nc():
    nc = bacc.Bacc(
        "TRN2", target_bir_lowering=False, debug=False, enable_asserts=False
    )
    xT = nc.dram_tensor("xT", [E, S], DT, kind="ExternalInput").ap()
    wq = nc.dram_tensor("wq", [E, EH], DT, kind="ExternalInput").ap()
    wk = nc.dram_tensor("wk", [E, EH], DT, kind="ExternalInput").ap()
    wv = nc.dram_tensor("wv", [E, EH], DT, kind="ExternalInput").ap()
    wo = nc.dram_tensor("wo", [EH, E], DT, kind="ExternalInput").ap()
    bq = nc.dram_tensor("bq", [EH], F32, kind="ExternalInput").ap()
    y = nc.dram_tensor("y", [S, E], F32, kind="ExternalOutput").ap()

    with tile.TileContext(nc) as tc:
        with (
            tc.tile_pool(name="consts", bufs=1) as consts,
            tc.tile_pool(name="work", bufs=4) as work,
            tc.tile_pool(name="norm", bufs=2) as norm,
            tc.tile_pool(name="outsb", bufs=2) as outsb,
            tc.tile_pool(name="psA", bufs=2, space="PSUM") as psA,
            tc.tile_pool(name="psO", bufs=2, space="PSUM") as psO,
            tc.tile_pool(name="dram", bufs=2, space="DRAM") as dram,
        ):
            # ---- constant loads ----
            xT_sb = consts.tile([P, EC, S], DT)
            xT_r = xT.rearrange("(c p) s -> c p s", p=P)
            for ec in range(EC):
                nc.sync.dma_start(out=xT_sb[:, ec, :], in_=xT_r[ec])
            wq_sb = consts.tile([P, EC, EH], DT)
            nc.sync.dma_start(out=wq_sb, in_=wq.rearrange("(c p) n -> p c n", p=P))
            wk_sb = consts.tile([P, EC, EH], DT)
            nc.sync.dma_start(out=wk_sb, in_=wk.rearrange("(c p) n -> p c n", p=P))
            wv_sb = consts.tile([P, EC, EH], DT)
            nc.sync.dma_start(out=wv_sb, in_=wv.rearrange("(c p) n -> p c n", p=P))
            wo_sb = consts.tile([P, MC, E], DT)
            nc.sync.dma_start(out=wo_sb, in_=wo.rearrange("(m p) n -> p m n", p=P))
            bq_sb = consts.tile([P, MC], F32)
            nc.sync.dma_start(out=bq_sb, in_=bq.rearrange("(m p) -> p m", p=P))


            QT_sb = consts.tile([P, MC, S], DT)
            KT_sb = consts.tile([P, MC, S], DT)
            V_sb = consts.tile([P, NT, HPC, D + 1], DT)
            OT_sb = consts.tile([P, MC, S], DT)
            nc.vector.memset(V_sb[:, :, :, D : D + 1], 1.0)

            # ---- QKV projections ----
            # K first, then V, then Q -- attention on the first q-chunk can
            # start as soon as Q's first half is done. All evacuations on DVE
            # (tensor_scalar adds bq per-partition) so ACT is free for exp.
            def qk_chunk(w_sb, dst, mc, t4, is_q):
                sl = bass.ts(t4, 512)
                ps = psA.tile(
                    [P, 512], F32, tag="big", name=f"qk{t4}{mc}{int(is_q)}"
                )
                for ec in range(EC):
                    nc.tensor.matmul(
                        ps,
                        lhsT=w_sb[:, ec, mc * P : (mc + 1) * P],
                        rhs=xT_sb[:, ec, sl],
                        start=(ec == 0),
                        stop=(ec == EC - 1),
                    )
                if is_q:
                    nc.vector.tensor_scalar_add(
                        out=dst[:, mc, sl], in0=ps, scalar1=bq_sb[:, mc : mc + 1]
                    )
                else:
                    nc.vector.tensor_copy(out=dst[:, mc, sl], in_=ps)

            def v_tile(t):
                ps = psA.tile([P, EH], F32, tag="big", name=f"v{t}")
                for ec in range(EC):
                    nc.tensor.matmul(
                        ps,
                        lhsT=xT_sb[:, ec, bass.ts(t, P)],
                        rhs=wv_sb[:, ec, :],
                        start=(ec == 0),
                        stop=(ec == EC - 1),
                    )
                nc.vector.tensor_copy(
                    out=V_sb[:, t, :, 0:D],
                    in_=ps.rearrange("p (h d) -> p h d", h=HPC),
                )

            for t4 in range(S // 512):
                for mc in range(MC):
                    qk_chunk(wk_sb, KT_sb, mc, t4, False)
                for t in range(4 * t4, 4 * t4 + 4):
                    v_tile(t)
            for t4 in range(S // 512):
                for mc in range(MC):
                    qk_chunk(wq_sb, QT_sb, mc, t4, True)

            # ---- attention + output projection, software pipelined ----
            y_r = y.rearrange("(t p) n -> t p n", p=P)

            def att_unit(hp, iq, Ou, Rs):
                """Scores^T -> exp -> [V|1]^T @ P^T for heads (2hp, 2hp+1) on
                q-chunk iq; evacuates unnormalized O^T + row sums to SBUF."""
                q0 = iq * QH
                O_pair = [
                    psO.tile([D + 1, QH], F32, tag="acc", name=f"O{hp}{iq}a"),
                    psO.tile([D + 1, QH], F32, tag="acc", name=f"O{hp}{iq}b"),
                ]
                for kt in range(NT):
                    ST_pair = [
                        psA.tile([P, QH], F32, tag="big", name=f"ST{hp}{iq}{kt}a"),
                        psA.tile([P, QH], F32, tag="big", name=f"ST{hp}{iq}{kt}b"),
                    ]
                    # scores^T: row-group packed pair (bases 0 / 64)
                    for qs in range(QH // 512):
                        for i, base in ((0, 0), (1, 64)):
                            nc.tensor.matmul(
                                ST_pair[i][:, bass.ts(qs, 512)],
                                lhsT=KT_sb[base : base + 64, hp, bass.ts(kt, P)],
                                rhs=QT_sb[
                                    base : base + 64,
                                    hp,
                                    q0 + qs * 512 : q0 + (qs + 1) * 512,
                                ],
                                start=True,
                                stop=True,
                            )
                    for i in range(2):
                        h_local = 2 * hp + i
                        PT = work.tile([P, QH], DT, tag="pt", name=f"PT{hp}{iq}{kt}{i}")
                        nc.scalar.activation(
                            out=PT, in_=ST_pair[i], func=AF.Exp, scale=SCALE
                        )
                        for qs in range(QH // 512):
                            nc.tensor.matmul(
                                O_pair[i][:, bass.ts(qs, 512)],
                                lhsT=V_sb[:, kt, h_local, :],
                                rhs=PT[:, bass.ts(qs, 512)],
                                start=(kt == 0),
                                stop=(kt == NT - 1),
                            )
                # fast psum evacuation: unnormalized O + row sums
                for i in range(2):
                    ou = work.tile([64, QH], F32, tag="ou", name=f"ou{hp}{iq}{i}")
                    nc.vector.tensor_copy(out=ou, in_=O_pair[i][0:D, :])
                    rsrow = norm.tile(
                        [1, QH], F32, tag="rs", bufs=4, name=f"rs{hp}{iq}{i}"
                    )
                    nc.vector.tensor_copy(out=rsrow, in_=O_pair[i][D : D + 1, :])
                    Ou.append(ou)
                    Rs.append(rsrow)

            def normalize(iq, Ou, Rs):
                """Batched approx-recip + DMA broadcast + DVE renorm into OT_sb."""
                q0 = iq * QH
                rdram = dram.tile([4, QH], F32, tag="rdram", name=f"rd{iq}")
                for u in range(4):
                    rc = norm.tile([1, QH], F32, tag="rc", bufs=4, name=f"rc{iq}{u}")
                    nc.vector.reciprocal_approx_fast(out=rc, in_=Rs[u])
                    nc.sync.dma_start(out=rdram[u : u + 1, :], in_=rc)
                bc = norm.tile([64, 4, QH], F32, tag="bc", name=f"bc{iq}")
                rdram_b = bass.AP(
                    tensor=rdram.tensor,
                    offset=rdram.offset,
                    ap=[[0, 64]] + list(rdram.ap),
                )
                nc.sync.dma_start(out=bc, in_=rdram_b)
                for u, (hp, i) in enumerate(((0, 0), (0, 1), (1, 0), (1, 1))):
                    nc.vector.tensor_mul(
                        out=OT_sb[64 * i : 64 * i + 64, hp, q0 : q0 + QH],
                        in0=Ou[u],
                        in1=bc[:, u, :],
                    )

            def y_proj(iq):
                for t in range(iq * (NT // NQH), (iq + 1) * (NT // NQH)):
                    psY = psA.tile([P, E], F32, tag="big", name=f"psY{t}")
                    for n2 in range(E // 512):
                        for mc in range(MC):
                            nc.tensor.matmul(
                                psY[:, bass.ts(n2, 512)],
                                lhsT=OT_sb[:, mc, bass.ts(t, P)],
                                rhs=wo_sb[:, mc, bass.ts(n2, 512)],
                                start=(mc == 0),
                                stop=(mc == MC - 1),
                            )
                    y_sb = outsb.tile([P, E], F32, tag="ysb", name=f"ysb{t}")
                    nc.vector.tensor_copy(out=y_sb, in_=psY)
                    nc.sync.dma_start(out=y_r[t], in_=y_sb)

            # Pipeline: Y(iq) is emitted after att(0, iq+1) so the PE has a
            # full unit of attention work queued before it reaches Y's
            # dependency on the normalize chain (engines run in order).
            state = {}
            for iq in range(NQH):
                Ou, Rs = [], []
                for hp in range(MC):
                    att_unit(hp, iq, Ou, Rs)
                    if hp == 0 and iq > 0:
                        y_proj(iq - 1)
                normalize(iq, Ou, Rs)
            y_proj(NQH - 1)

    nc.compile()
    return nc


_NC_CACHE = {}


def get_nc():
    if "nc" not in _NC_CACHE:
        _NC_CACHE["nc"] = build_nc()
    return _NC_CACHE["nc"]


def make_in_maps(x, Wq, bq, Wk, Wv, Wo):
    xT_by_batch = [
        np.ascontiguousarray(x[b].T).astype(NP_DT) for b in range(B)
    ]
    in_maps = []
    for c in range(NCORES):
        b, hg = divmod(c, NCORES // B)
        hs = slice(hg * EH, (hg + 1) * EH)
        in_maps.append(
            {
                "xT": xT_by_batch[b],
                "wq": np.ascontiguousarray(Wq[:, hs]).astype(NP_DT),
                "wk": np.ascontiguousarray(Wk[:, hs]).astype(NP_DT),
                "wv": np.ascontiguousarray(Wv[:, hs]).astype(NP_DT),
                "wo": np.ascontiguousarray(Wo[hs, :]).astype(NP_DT),
                "bq": np.ascontiguousarray(bq[hs]).astype(np.float32),
            }
        )
    return in_maps


def gather_out(results, bv, Wo, bo):
    bias_row = (
        bv.astype(np.float64) @ Wo.astype(np.float64) + bo.astype(np.float64)
    ).astype(np.float32)
    out = np.empty((B, S, E), np.float32)
    gpb = NCORES // B
    for b in range(B):
        acc = results[gpb * b]["y"].copy()
        for i in range(1, gpb):
            acc += results[gpb * b + i]["y"]
        out[b] = acc + bias_row
    return out


def kernel(x, Wq, bq, Wk, bk, Wv, bv, Wo, bo, **_):
    x = np.asarray(x, np.float32)
    nc = get_nc()
    in_maps = make_in_maps(
        x,
        np.asarray(Wq, np.float32),
        np.asarray(bq, np.float32),
        np.asarray(Wk, np.float32),
        np.asarray(Wv, np.float32),
        np.asarray(Wo, np.float32),
    )
    res = run_bass_kernel_spmd(nc, in_maps, list(range(NCORES)))
    return gather_out(
        res.results, np.asarray(bv, np.float32), np.asarray(Wo, np.float32),
        np.asarray(bo, np.float32)
    )


# revision 16
# speedup vs baseline: 1.0477x; 1.0477x over previous
"""Multi-head self-attention (B=2, S=2048, E=1024, H=16, D=64) on 8 TRN2 cores.

Sharding: tensor-parallel over (batch, head-group): core c handles batch c//4
and heads [4*(c%4), 4*(c%4)+4). Each core computes its heads' attention output
projected through its slice of Wo; the host sums the 4 partial outputs per
batch and adds the constant bias row (bv @ Wo + bo).

Device-side math (per core, transposed formulation so no transposes needed):
  QT = Wq_c^T @ x^T + bq_c        [256, S]   (bias bk dropped: softmax-invariant)
  KT = Wk_c^T @ x^T               [256, S]
  V  = x @ Wv_c                   [S, 256]   (bias bv folded into host bias row)
  S^T tile = K @ Q^T              (PE, per 128-k-token x 1024-q tile)
  P^T = exp(S^T / 8)              (ACT, no max subtraction: scores ~ N(0,1))
  O^T aug = [V | 1]^T @ P^T       (PE, accumulated over k tiles; row 64 = sum)
  O^T = O^T aug[0:64] / row 64    (recip + PE broadcast + DVE mul)
  Y = O @ Wo_c                    [S, 1024]  fp32 partial out
"""

import numpy as np
import ml_dtypes

import concourse.bass as bass
import concourse.bacc as bacc
import concourse.tile as tile
from concourse import mybir
from concourse.bass_utils import run_bass_kernel_spmd

B, S, E = 2, 2048, 1024
H, D = 16, 64
NCORES = 8
HPC = 4                 # heads per core
EH = HPC * D            # 256: per-core head width
P = 128
EC = E // P             # 8 E-chunks of 128
MC = EH // P            # 2 Eh-chunks of 128
NT = S // P             # 16 token tiles of 128
QH = 1024               # q-chunk processed per attention unit
NQH = S // QH           # 2
SCALE = 1.0 / float(np.sqrt(D))

DT = mybir.dt.bfloat16
NP_DT = ml_dtypes.bfloat16
F32 = mybir.dt.float32
F32R = mybir.dt.float32r

AF = mybir.ActivationFunctionType


def build_nc():
    nc = bacc.Bacc(
        "TRN2", target_bir_lowering=False, debug=False, enable_asserts=False
    )
    xT = nc.dram_tensor("xT", [E, S], DT, kind="ExternalInput").ap()
    wq = nc.dram_tensor("wq", [E, EH], DT, kind="ExternalInput").ap()
    wk = nc.dram_tensor("wk", [E, EH], DT, kind="ExternalInput").ap()
    wv = nc.dram_tensor("wv", [E, EH], DT, kind="ExternalInput").ap()
    wo = nc.dram_tensor("wo", [EH, E], DT, kind="ExternalInput").ap()
    bq = nc.dram_tensor("bq", [EH], F32, kind="ExternalInput").ap()
    y = nc.dram_tensor("y", [S, E], F32, kind="ExternalOutput").ap()

    with tile.TileContext(nc) as tc:
        with (
            tc.tile_pool(name="consts", bufs=1) as consts,
            tc.tile_pool(name="work", bufs=4) as work,
            tc.tile_pool(name="norm", bufs=2) as norm,
            tc.tile_pool(name="outsb", bufs=2) as outsb,
            tc.tile_pool(name="psA", bufs=2, space="PSUM") as psA,
            tc.tile_pool(name="psO", bufs=2, space="PSUM") as psO,
            tc.tile_pool(name="dram", bufs=2, space="DRAM") as dram,
        ):
            # ---- constant loads ----
            wk_sb = consts.tile([P, EC, EH], DT)
            nc.gpsimd.dma_start(out=wk_sb, in_=wk.rearrange("(c p) n -> p c n", p=P))
            wv_sb = consts.tile([P, EC, EH], DT)
            nc.gpsimd.dma_start(out=wv_sb, in_=wv.rearrange("(c p) n -> p c n", p=P))
            xT_sb = consts.tile([P, EC, S], DT)
            xT_r = xT.rearrange("(c p) s -> c p s", p=P)
            for ec in range(EC):
                nc.sync.dma_start(out=xT_sb[:, ec, :], in_=xT_r[ec])
            wq_sb = consts.tile([P, EC, EH], DT)
            nc.scalar.dma_start(out=wq_sb, in_=wq.rearrange("(c p) n -> p c n", p=P))
            wo_sb = consts.tile([P, MC, E], DT)
            nc.scalar.dma_start(out=wo_sb, in_=wo.rearrange("(m p) n -> p m n", p=P))
            bq_sb = consts.tile([P, MC], F32)
            nc.gpsimd.dma_start(out=bq_sb, in_=bq.rearrange("(m p) -> p m", p=P))


            QT_sb = consts.tile([P, MC, S], DT)
            KT_sb = consts.tile([P, MC, S], DT)
            V_sb = consts.tile([P, NT, HPC, D + 1], DT)
            OT_sb = consts.tile([P, MC, S], DT)
            nc.vector.memset(V_sb[:, :, :, D : D + 1], 1.0)

            # ---- QKV projections ----
            # K first, then V, then Q -- attention on the first q-chunk can
            # start as soon as Q's first half is done. All evacuations on DVE
            # (tensor_scalar adds bq per-partition) so ACT is free for exp.
            def qk_chunk(w_sb, dst, mc, t4, is_q):
                sl = bass.ts(t4, 512)
                ps = psA.tile(
                    [P, 512], F32, tag="big", name=f"qk{t4}{mc}{int(is_q)}"
                )
                for ec in range(EC):
                    nc.tensor.matmul(
                        ps,
                        lhsT=w_sb[:, ec, mc * P : (mc + 1) * P],
                        rhs=xT_sb[:, ec, sl],
                        start=(ec == 0),
                        stop=(ec == EC - 1),
                    )
                if is_q:
                    nc.vector.tensor_scalar_add(
                        out=dst[:, mc, sl], in0=ps, scalar1=bq_sb[:, mc : mc + 1]
                    )
                else:
                    nc.vector.tensor_copy(out=dst[:, mc, sl], in_=ps)

            def v_tile(t):
                ps = psA.tile([P, EH], F32, tag="big", name=f"v{t}")
                for ec in range(EC):
                    nc.tensor.matmul(
                        ps,
                        lhsT=xT_sb[:, ec, bass.ts(t, P)],
                        rhs=wv_sb[:, ec, :],
                        start=(ec == 0),
                        stop=(ec == EC - 1),
                    )
                nc.vector.tensor_copy(
                    out=V_sb[:, t, :, 0:D],
                    in_=ps.rearrange("p (h d) -> p h d", h=HPC),
                )

            for t4 in range(S // 512):
                for mc in range(MC):
                    qk_chunk(wk_sb, KT_sb, mc, t4, False)
                for t in range(4 * t4, 4 * t4 + 4):
                    v_tile(t)
            for t4 in range(2):
                for mc in range(MC):
                    qk_chunk(wq_sb, QT_sb, mc, t4, True)

            # ---- attention + output projection, software pipelined ----
            y_r = y.rearrange("(t p) n -> t p n", p=P)

            def att_unit(hp, iq, Ou, Rs, deferred=()):
                """Scores^T -> exp -> [V|1]^T @ P^T for heads (2hp, 2hp+1) on
                q-chunk iq; evacuates unnormalized O^T + row sums to SBUF.
                `deferred` maps kt -> emit-callback for pipelined fill work."""
                deferred = dict(deferred)
                q0 = iq * QH
                O_pair = [
                    psO.tile([D + 1, QH], F32, tag="acc", name=f"O{hp}{iq}a"),
                    psO.tile([D + 1, QH], F32, tag="acc", name=f"O{hp}{iq}b"),
                ]
                for kt in range(NT):
                    if kt in deferred:
                        deferred[kt]()
                    ST_pair = [
                        psA.tile([P, QH], F32, tag="big", name=f"ST{hp}{iq}{kt}a"),
                        psA.tile([P, QH], F32, tag="big", name=f"ST{hp}{iq}{kt}b"),
                    ]
                    # scores^T: row-group packed pair (bases 0 / 64)
                    for qs in range(QH // 512):
                        for i, base in ((0, 0), (1, 64)):
                            nc.tensor.matmul(
                                ST_pair[i][:, bass.ts(qs, 512)],
                                lhsT=KT_sb[base : base + 64, hp, bass.ts(kt, P)],
                                rhs=QT_sb[
                                    base : base + 64,
                                    hp,
                                    q0 + qs * 512 : q0 + (qs + 1) * 512,
                                ],
                                start=True,
                                stop=True,
                            )
                    for i in range(2):
                        h_local = 2 * hp + i
                        PT = work.tile([P, QH], DT, tag="pt", name=f"PT{hp}{iq}{kt}{i}")
                        nc.scalar.activation(
                            out=PT, in_=ST_pair[i], func=AF.Exp, scale=SCALE
                        )
                        for qs in range(QH // 512):
                            nc.tensor.matmul(
                                O_pair[i][:, bass.ts(qs, 512)],
                                lhsT=V_sb[:, kt, h_local, :],
                                rhs=PT[:, bass.ts(qs, 512)],
                                start=(kt == 0),
                                stop=(kt == NT - 1),
                            )
                # fast psum evacuation: unnormalized O + approx-recip of sums
                for i in range(2):
                    ou = work.tile([64, QH], F32, tag="ou", name=f"ou{hp}{iq}{i}")
                    nc.vector.tensor_copy(out=ou, in_=O_pair[i][0:D, :])
                    rs = norm.tile([1, QH], F32, tag="rs", bufs=4,
                                   name=f"rs{hp}{iq}{i}")
                    nc.vector.tensor_copy(out=rs, in_=O_pair[i][D : D + 1, :])
                    rc = norm.tile([1, QH], F32, tag="rc", bufs=4,
                                   name=f"rc{hp}{iq}{i}")
                    nc.vector.reciprocal_approx_fast(out=rc, in_=rs)
                    Ou.append(ou)
                    Rs.append(rc)

            def normalize(iq, Ou, Rs):
                """Batched approx-recip + DMA broadcast + DVE renorm into OT_sb."""
                q0 = iq * QH
                rdram = dram.tile([4, QH], F32, tag="rdram", name=f"rd{iq}")
                for u in range(4):
                    nc.sync.dma_start(out=rdram[u : u + 1, :], in_=Rs[u])
                bc = norm.tile([64, 4, QH], F32, tag="bc", name=f"bc{iq}")
                rdram_b = bass.AP(
                    tensor=rdram.tensor,
                    offset=rdram.offset,
                    ap=[[0, 64]] + list(rdram.ap),
                )
                nc.sync.dma_start(out=bc, in_=rdram_b)
                for u, (hp, i) in enumerate(((0, 0), (0, 1), (1, 0), (1, 1))):
                    nc.vector.tensor_mul(
                        out=OT_sb[64 * i : 64 * i + 64, hp, q0 : q0 + QH],
                        in0=Ou[u],
                        in1=bc[:, u, :],
                    )

            def y_tile(t):
                    psY = psA.tile([P, E], F32, tag="big", name=f"psY{t}")
                    for n2 in range(E // 512):
                        for mc in range(MC):
                            nc.tensor.matmul(
                                psY[:, bass.ts(n2, 512)],
                                lhsT=OT_sb[:, mc, bass.ts(t, P)],
                                rhs=wo_sb[:, mc, bass.ts(n2, 512)],
                                start=(mc == 0),
                                stop=(mc == MC - 1),
                            )
                    y_sb = outsb.tile([P, E], F32, tag="ysb", name=f"ysb{t}")
                    nc.vector.tensor_copy(out=y_sb, in_=psY)
                    nc.sync.dma_start(out=y_r[t], in_=y_sb)

            def y_proj(iq):
                for t in range(iq * (NT // NQH), (iq + 1) * (NT // NQH)):
                    y_tile(t)

            # Pipeline: Y(iq) is emitted after att(0, iq+1) so the PE has a
            # full unit of attention work queued before it reaches Y's
            # dependency on the normalize chain (engines run in order).
            for iq in range(NQH):
                Ou, Rs = [], []
                for hp in range(MC):
                    deferred = {}
                    if hp == 0 and iq == 0:
                        for j, t4 in enumerate((2, 3)):
                            for mc in range(MC):
                                deferred[2 + 2 * (2 * j + mc)] = (
                                    lambda w=wq_sb, d=QT_sb, m=mc, t=t4:
                                    qk_chunk(w, d, m, t, True)
                                )
                    if hp == 0 and iq > 0:
                        base_t = (iq - 1) * (NT // NQH)
                        for j in range(NT // NQH):
                            deferred[2 * j + 1] = (
                                lambda t=base_t + j: y_tile(t)
                            )
                    att_unit(hp, iq, Ou, Rs, deferred)
                normalize(iq, Ou, Rs)
            y_proj(NQH - 1)

    nc.compile()
    return nc


_NC_CACHE = {}


def get_nc():
    if "nc" not in _NC_CACHE:
        _NC_CACHE["nc"] = build_nc()
    return _NC_CACHE["nc"]


def make_in_maps(x, Wq, bq, Wk, Wv, Wo):
    xT_by_batch = [
        np.ascontiguousarray(x[b].T).astype(NP_DT) for b in range(B)
    ]
    in_maps = []
    for c in range(NCORES):
        b, hg = divmod(c, NCORES // B)
        hs = slice(hg * EH, (hg + 1) * EH)
        in_maps.append(
            {
                "xT": xT_by_batch[b],
                "wq": np.ascontiguousarray(Wq[:, hs]).astype(NP_DT),
                "wk": np.ascontiguousarray(Wk[:, hs]).astype(NP_DT),
                "wv": np.ascontiguousarray(Wv[:, hs]).astype(NP_DT),
                "wo": np.ascontiguousarray(Wo[hs, :]).astype(NP_DT),
                "bq": np.ascontiguousarray(bq[hs]).astype(np.float32),
            }
        )
    return in_maps


def gather_out(results, bv, Wo, bo):
    bias_row = (
        bv.astype(np.float64) @ Wo.astype(np.float64) + bo.astype(np.float64)
    ).astype(np.float32)
    out = np.empty((B, S, E), np.float32)
    gpb = NCORES // B
    for b in range(B):
        acc = results[gpb * b]["y"].copy()
        for i in range(1, gpb):
            acc += results[gpb * b + i]["y"]
        out[b] = acc + bias_row
    return out


def kernel(x, Wq, bq, Wk, bk, Wv, bv, Wo, bo, **_):
    x = np.asarray(x, np.float32)
    nc = get_nc()
    in_maps = make_in_maps(
        x,
        np.asarray(Wq, np.float32),
        np.asarray(bq, np.float32),
        np.asarray(Wk, np.float32),
        np.asarray(Wv, np.float32),
        np.asarray(Wo, np.float32),
    )
    res = run_bass_kernel_spmd(nc, in_maps, list(range(NCORES)))
    return gather_out(
        res.results, np.asarray(bv, np.float32), np.asarray(Wo, np.float32),
        np.asarray(bo, np.float32)
    )


# revision 17
# speedup vs baseline: 1.1177x; 1.0668x over previous
"""Multi-head self-attention (B=2, S=2048, E=1024, H=16, D=64) on 8 TRN2 cores.

Sharding: tensor-parallel over (batch, head-group): core c handles batch c//4
and heads [4*(c%4), 4*(c%4)+4). Each core computes its heads' attention output
projected through its slice of Wo; the host sums the 4 partial outputs per
batch and adds the constant bias row (bv @ Wo + bo).

Device-side math (per core, transposed formulation so no transposes needed):
  QT = Wq_c^T @ x^T + bq_c        [256, S]   (bias bk dropped: softmax-invariant)
  KT = Wk_c^T @ x^T               [256, S]
  V  = x @ Wv_c                   [S, 256]   (bias bv folded into host bias row)
  S^T tile = K @ Q^T              (PE, per 128-k-token x 1024-q tile)
  P^T = exp(S^T / 8)              (ACT, no max subtraction: scores ~ N(0,1))
  O^T aug = [V | 1]^T @ P^T       (PE, accumulated over k tiles; row 64 = sum)
  O^T = O^T aug[0:64] / row 64    (recip + PE broadcast + DVE mul)
  Y = O @ Wo_c                    [S, 1024]  fp32 partial out
"""

import numpy as np
import ml_dtypes

import concourse.bass as bass
import concourse.bacc as bacc
import concourse.tile as tile
from concourse import mybir
from concourse.bass_utils import run_bass_kernel_spmd

B, S, E = 2, 2048, 1024
H, D = 16, 64
NCORES = 8
HPC = 4                 # heads per core
EH = HPC * D            # 256: per-core head width
P = 128
EC = E // P             # 8 E-chunks of 128
MC = EH // P            # 2 Eh-chunks of 128
NT = S // P             # 16 token tiles of 128
QH = 1024               # q-chunk processed per attention unit
NQH = S // QH           # 2
SCALE = 1.0 / float(np.sqrt(D))

DT = mybir.dt.bfloat16
NP_DT = ml_dtypes.bfloat16
F32 = mybir.dt.float32
F32R = mybir.dt.float32r

AF = mybir.ActivationFunctionType


def build_nc():
    nc = bacc.Bacc(
        "TRN2", target_bir_lowering=False, debug=False, enable_asserts=False
    )
    xT = nc.dram_tensor("xT", [E, S], DT, kind="ExternalInput").ap()
    wq = nc.dram_tensor("wq", [E, EH], DT, kind="ExternalInput").ap()
    wk = nc.dram_tensor("wk", [E, EH], DT, kind="ExternalInput").ap()
    wv = nc.dram_tensor("wv", [E, EH], DT, kind="ExternalInput").ap()
    wo = nc.dram_tensor("wo", [EH, E], DT, kind="ExternalInput").ap()
    bq = nc.dram_tensor("bq", [EH], F32, kind="ExternalInput").ap()
    y = nc.dram_tensor("y", [S, E], F32, kind="ExternalOutput").ap()

    with tile.TileContext(nc) as tc:
        with (
            tc.tile_pool(name="consts", bufs=1) as consts,
            tc.tile_pool(name="work", bufs=4) as work,
            tc.tile_pool(name="norm", bufs=2) as norm,
            tc.tile_pool(name="outsb", bufs=2) as outsb,
            tc.tile_pool(name="psA", bufs=2, space="PSUM") as psA,
            tc.tile_pool(name="psO", bufs=2, space="PSUM") as psO,
            tc.tile_pool(name="dram", bufs=2, space="DRAM") as dram,
        ):
            # ---- constant loads ----
            wk_sb = consts.tile([P, EC, EH], DT)
            nc.gpsimd.dma_start(out=wk_sb, in_=wk.rearrange("(c p) n -> p c n", p=P))
            wv_sb = consts.tile([P, EC, EH], DT)
            nc.gpsimd.dma_start(out=wv_sb, in_=wv.rearrange("(c p) n -> p c n", p=P))
            xT_sb = consts.tile([P, EC, S], DT)
            xT_r = xT.rearrange("(c p) s -> c p s", p=P)
            for ec in range(EC):
                nc.sync.dma_start(out=xT_sb[:, ec, :], in_=xT_r[ec])
            wq_sb = consts.tile([P, EC, EH], DT)
            nc.scalar.dma_start(out=wq_sb, in_=wq.rearrange("(c p) n -> p c n", p=P))
            wo_sb = consts.tile([P, MC, E], DT)
            nc.scalar.dma_start(out=wo_sb, in_=wo.rearrange("(m p) n -> p m n", p=P))
            bq_sb = consts.tile([P, MC], F32)
            nc.gpsimd.dma_start(out=bq_sb, in_=bq.rearrange("(m p) -> p m", p=P))


            QT_sb = consts.tile([P, MC, S], DT)
            KT_sb = consts.tile([P, MC, S], DT)
            V_sb = consts.tile([P, NT, HPC, D + 1], DT)
            OT_sb = consts.tile([P, MC, S], DT)
            nc.vector.memset(V_sb[:, :, :, D : D + 1], 1.0)

            # ---- QKV projections ----
            # K first, then V, then Q -- attention on the first q-chunk can
            # start as soon as Q's first half is done. All evacuations on DVE
            # (tensor_scalar adds bq per-partition) so ACT is free for exp.
            def qk_chunk(w_sb, dst, mc, t4, is_q):
                sl = bass.ts(t4, 512)
                ps = psA.tile(
                    [P, 512], F32, tag="big", name=f"qk{t4}{mc}{int(is_q)}"
                )
                for ec in range(EC):
                    nc.tensor.matmul(
                        ps,
                        lhsT=w_sb[:, ec, mc * P : (mc + 1) * P],
                        rhs=xT_sb[:, ec, sl],
                        start=(ec == 0),
                        stop=(ec == EC - 1),
                    )
                if is_q:
                    nc.vector.tensor_scalar_add(
                        out=dst[:, mc, sl], in0=ps, scalar1=bq_sb[:, mc : mc + 1]
                    )
                else:
                    nc.vector.tensor_copy(out=dst[:, mc, sl], in_=ps)

            def v_tile(t):
                ps = psA.tile([P, EH], F32, tag="big", name=f"v{t}")
                for ec in range(EC):
                    nc.tensor.matmul(
                        ps,
                        lhsT=xT_sb[:, ec, bass.ts(t, P)],
                        rhs=wv_sb[:, ec, :],
                        start=(ec == 0),
                        stop=(ec == EC - 1),
                    )
                nc.vector.tensor_copy(
                    out=V_sb[:, t, :, 0:D],
                    in_=ps.rearrange("p (h d) -> p h d", h=HPC),
                )

            for t4 in range(S // 512):
                for mc in range(MC):
                    qk_chunk(wk_sb, KT_sb, mc, t4, False)
                for t in range(4 * t4, 4 * t4 + 4):
                    v_tile(t)
            for t4 in range(2):
                for mc in range(MC):
                    qk_chunk(wq_sb, QT_sb, mc, t4, True)

            # ---- attention + output projection, software pipelined ----
            y_r = y.rearrange("(t p) n -> t p n", p=P)

            def att_unit(hp, iq, Ou, Rs, deferred=()):
                """Scores^T -> exp -> [V|1]^T @ P^T for heads (2hp, 2hp+1) on
                q-chunk iq; evacuates unnormalized O^T + row sums to SBUF.
                `deferred` maps kt -> emit-callback for pipelined fill work."""
                deferred = dict(deferred)
                q0 = iq * QH
                O_pair = [
                    psO.tile([D + 1, QH], F32, tag="acc", name=f"O{hp}{iq}a"),
                    psO.tile([D + 1, QH], F32, tag="acc", name=f"O{hp}{iq}b"),
                ]
                for kt in range(NT):
                    if kt in deferred:
                        deferred[kt]()
                    ST_pair = [
                        psA.tile([P, QH], F32, tag="big", name=f"ST{hp}{iq}{kt}a"),
                        psA.tile([P, QH], F32, tag="big", name=f"ST{hp}{iq}{kt}b"),
                    ]
                    # scores^T: row-group packed pair (bases 0 / 64)
                    for qs in range(QH // 512):
                        for i, base in ((0, 0), (1, 64)):
                            nc.tensor.matmul(
                                ST_pair[i][:, bass.ts(qs, 512)],
                                lhsT=KT_sb[base : base + 64, hp, bass.ts(kt, P)],
                                rhs=QT_sb[
                                    base : base + 64,
                                    hp,
                                    q0 + qs * 512 : q0 + (qs + 1) * 512,
                                ],
                                start=True,
                                stop=True,
                            )
                    for i in range(2):
                        h_local = 2 * hp + i
                        PT = work.tile([P, QH], DT, tag="pt", name=f"PT{hp}{iq}{kt}{i}")
                        nc.scalar.activation(
                            out=PT, in_=ST_pair[i], func=AF.Exp, scale=SCALE
                        )
                        for qs in range(QH // 512):
                            nc.tensor.matmul(
                                O_pair[i][:, bass.ts(qs, 512)],
                                lhsT=V_sb[:, kt, h_local, :],
                                rhs=PT[:, bass.ts(qs, 512)],
                                start=(kt == 0),
                                stop=(kt == NT - 1),
                            )
                # fast psum evacuation: unnormalized O + approx-recip of sums
                for i in range(2):
                    ou = work.tile([64, QH], F32, tag="ou", name=f"ou{hp}{iq}{i}")
                    nc.vector.tensor_copy(out=ou, in_=O_pair[i][0:D, :])
                    rs = norm.tile([1, QH], F32, tag="rs", bufs=4,
                                   name=f"rs{hp}{iq}{i}")
                    nc.vector.tensor_copy(out=rs, in_=O_pair[i][D : D + 1, :])
                    rc = norm.tile([1, QH], F32, tag="rc", bufs=4,
                                   name=f"rc{hp}{iq}{i}")
                    nc.vector.reciprocal_approx_fast(out=rc, in_=rs)
                    Ou.append(ou)
                    Rs.append(rc)

            def normalize(iq, hp, Ou, Rs):
                """Approx-recip rows -> DMA broadcast -> DVE renorm into OT_sb
                for one (iq, hp) unit; runs concurrently with the next unit."""
                q0 = iq * QH
                rdram = dram.tile([2, QH], F32, tag="rdram", bufs=4,
                                  name=f"rd{iq}{hp}")
                for i in range(2):
                    nc.sync.dma_start(out=rdram[i : i + 1, :], in_=Rs[i])
                bc = norm.tile([64, 2, QH], F32, tag="bc", bufs=4,
                               name=f"bc{iq}{hp}")
                rdram_b = bass.AP(
                    tensor=rdram.tensor,
                    offset=rdram.offset,
                    ap=[[0, 64]] + list(rdram.ap),
                )
                nc.sync.dma_start(out=bc, in_=rdram_b)
                for i in range(2):
                    nc.vector.tensor_mul(
                        out=OT_sb[64 * i : 64 * i + 64, hp, q0 : q0 + QH],
                        in0=Ou[i],
                        in1=bc[:, i, :],
                    )

            def y_tile(t):
                    psY = psA.tile([P, E], F32, tag="big", name=f"psY{t}")
                    for n2 in range(E // 512):
                        for mc in range(MC):
                            nc.tensor.matmul(
                                psY[:, bass.ts(n2, 512)],
                                lhsT=OT_sb[:, mc, bass.ts(t, P)],
                                rhs=wo_sb[:, mc, bass.ts(n2, 512)],
                                start=(mc == 0),
                                stop=(mc == MC - 1),
                            )
                    y_sb = outsb.tile([P, E], F32, tag="ysb", name=f"ysb{t}")
                    nc.vector.tensor_copy(out=y_sb, in_=psY)
                    nc.sync.dma_start(out=y_r[t], in_=y_sb)

            def y_proj(iq):
                for t in range(iq * (NT // NQH), (iq + 1) * (NT // NQH)):
                    y_tile(t)

            # Pipeline: Y(iq) is emitted after att(0, iq+1) so the PE has a
            # full unit of attention work queued before it reaches Y's
            # dependency on the normalize chain (engines run in order).
            for iq in range(NQH):
                for hp in range(MC):
                    Ou, Rs = [], []
                    deferred = {}
                    if hp == 0 and iq == 0:
                        for j, t4 in enumerate((2, 3)):
                            for mc in range(MC):
                                deferred[2 + 2 * (2 * j + mc)] = (
                                    lambda w=wq_sb, d=QT_sb, m=mc, t=t4:
                                    qk_chunk(w, d, m, t, True)
                                )
                    if hp == 0 and iq > 0:
                        base_t = (iq - 1) * (NT // NQH)
                        for j in range(NT // NQH):
                            deferred[6 + j] = (
                                lambda t=base_t + j: y_tile(t)
                            )
                    att_unit(hp, iq, Ou, Rs, deferred)
                    normalize(iq, hp, Ou, Rs)
            y_proj(NQH - 1)

    nc.compile()
    return nc


_NC_CACHE = {}


def get_nc():
    if "nc" not in _NC_CACHE:
        _NC_CACHE["nc"] = build_nc()
    return _NC_CACHE["nc"]


def make_in_maps(x, Wq, bq, Wk, Wv, Wo):
    xT_by_batch = [
        np.ascontiguousarray(x[b].T).astype(NP_DT) for b in range(B)
    ]
    in_maps = []
    for c in range(NCORES):
        b, hg = divmod(c, NCORES // B)
        hs = slice(hg * EH, (hg + 1) * EH)
        in_maps.append(
            {
                "xT": xT_by_batch[b],
                "wq": np.ascontiguousarray(Wq[:, hs]).astype(NP_DT),
                "wk": np.ascontiguousarray(Wk[:, hs]).astype(NP_DT),
                "wv": np.ascontiguousarray(Wv[:, hs]).astype(NP_DT),
                "wo": np.ascontiguousarray(Wo[hs, :]).astype(NP_DT),
                "bq": np.ascontiguousarray(bq[hs]).astype(np.float32),
            }
        )
    return in_maps


def gather_out(results, bv, Wo, bo):
    bias_row = (
        bv.astype(np.float64) @ Wo.astype(np.float64) + bo.astype(np.float64)
    ).astype(np.float32)
    out = np.empty((B, S, E), np.float32)
    gpb = NCORES // B
    for b in range(B):
        acc = results[gpb * b]["y"].copy()
        for i in range(1, gpb):
            acc += results[gpb * b + i]["y"]
        out[b] = acc + bias_row
    return out


def kernel(x, Wq, bq, Wk, bk, Wv, bv, Wo, bo, **_):
    x = np.asarray(x, np.float32)
    nc = get_nc()
    in_maps = make_in_maps(
        x,
        np.asarray(Wq, np.float32),
        np.asarray(bq, np.float32),
        np.asarray(Wk, np.float32),
        np.asarray(Wv, np.float32),
        np.asarray(Wo, np.float32),
    )
    res = run_bass_kernel_spmd(nc, in_maps, list(range(NCORES)))
    return gather_out(
        res.results, np.asarray(bv, np.float32), np.asarray(Wo, np.float32),
        np.asarray(bo, np.float32)
    )


# revision 22
# speedup vs baseline: 1.1344x; 1.0149x over previous
"""Multi-head self-attention (B=2, S=2048, E=1024, H=16, D=64) on 8 TRN2 cores.

Sharding: tensor-parallel over (batch, head-group): core c handles batch c//4
and heads [4*(c%4), 4*(c%4)+4). Each core computes its heads' attention output
projected through its slice of Wo; the host sums the 4 partial outputs per
batch and adds the constant bias row (bv @ Wo + bo).

Device-side math (per core, transposed formulation so no transposes needed):
  QT = Wq_c^T @ x^T + bq_c        [256, S]   (bias bk dropped: softmax-invariant)
  KT = Wk_c^T @ x^T               [256, S]
  V  = x @ Wv_c                   [S, 256]   (bias bv folded into host bias row)
  S^T tile = K @ Q^T              (PE, per 128-k-token x 1024-q tile)
  P^T = exp(S^T / 8)              (ACT, no max subtraction: scores ~ N(0,1))
  O^T aug = [V | 1]^T @ P^T       (PE, accumulated over k tiles; row 64 = sum)
  O^T = O^T aug[0:64] / row 64    (recip + PE broadcast + DVE mul)
  Y = O @ Wo_c                    [S, 1024]  fp32 partial out
"""

import numpy as np
import ml_dtypes

import concourse.bass as bass
import concourse.bacc as bacc
import concourse.tile as tile
from concourse import mybir
from concourse.bass_utils import run_bass_kernel_spmd

B, S, E = 2, 2048, 1024
H, D = 16, 64
NCORES = 8
HPC = 4                 # heads per core
EH = HPC * D            # 256: per-core head width
P = 128
EC = E // P             # 8 E-chunks of 128
MC = EH // P            # 2 Eh-chunks of 128
NT = S // P             # 16 token tiles of 128
QH = 1024               # q-chunk processed per attention unit
NQH = S // QH           # 2
SCALE = 1.0 / float(np.sqrt(D))
ESHIFT = -2.0           # exp(s/8 - 2): keeps P below fp8e4m3 max (448);
                        # cancels in the softmax normalization

DT = mybir.dt.bfloat16
NP_DT = ml_dtypes.bfloat16
F32 = mybir.dt.float32
F32R = mybir.dt.float32r
F8 = mybir.dt.float8e4
NTP = NT // 2           # kt pairs for DoubleRow mm2
VPAD = 72               # padded per-head V row (16B-aligned pair stride)

AF = mybir.ActivationFunctionType


def build_nc():
    nc = bacc.Bacc(
        "TRN2", target_bir_lowering=False, debug=False, enable_asserts=False
    )
    xT = nc.dram_tensor("xT", [E, S], DT, kind="ExternalInput").ap()
    wq = nc.dram_tensor("wq", [E, EH], DT, kind="ExternalInput").ap()
    wk = nc.dram_tensor("wk", [E, EH], DT, kind="ExternalInput").ap()
    wv = nc.dram_tensor("wv", [E, EH], DT, kind="ExternalInput").ap()
    wo = nc.dram_tensor("wo", [EH, E], DT, kind="ExternalInput").ap()
    bq = nc.dram_tensor("bq", [EH], F32, kind="ExternalInput").ap()
    y = nc.dram_tensor("y", [S, E], F32, kind="ExternalOutput").ap()

    with tile.TileContext(nc) as tc:
        with (
            tc.tile_pool(name="consts", bufs=1) as consts,
            tc.tile_pool(name="work", bufs=4) as work,
            tc.tile_pool(name="norm", bufs=2) as norm,
            tc.tile_pool(name="outsb", bufs=2) as outsb,
            tc.tile_pool(name="psA", bufs=2, space="PSUM") as psA,
            tc.tile_pool(name="psO", bufs=2, space="PSUM") as psO,
            tc.tile_pool(name="dram", bufs=2, space="DRAM") as dram,
        ):
            # ---- constant loads ----
            wk_sb = consts.tile([P, EC, EH], DT)
            nc.gpsimd.dma_start(out=wk_sb, in_=wk.rearrange("(c p) n -> p c n", p=P))
            wv_sb = consts.tile([P, EC, EH], DT)
            nc.gpsimd.dma_start(out=wv_sb, in_=wv.rearrange("(c p) n -> p c n", p=P))
            xT_sb = consts.tile([P, EC, S], DT)
            xT_r = xT.rearrange("(c p) s -> p c s", p=P)
            NSC = 8
            for sc_i in range(NSC):
                ssl = bass.ts(sc_i, S // NSC)
                nc.sync.dma_start(out=xT_sb[:, :, ssl], in_=xT_r[:, :, ssl])
            wq_sb = consts.tile([P, EC, EH], DT)
            nc.scalar.dma_start(out=wq_sb, in_=wq.rearrange("(c p) n -> p c n", p=P))
            wo_sb = consts.tile([P, MC, E], DT)
            nc.scalar.dma_start(out=wo_sb, in_=wo.rearrange("(m p) n -> p m n", p=P))
            bq_sb = consts.tile([P, MC], F32)
            nc.gpsimd.dma_start(out=bq_sb, in_=bq.rearrange("(m p) -> p m", p=P))


            eshift_sb = consts.tile([P, 1], F32)
            nc.vector.memset(eshift_sb, ESHIFT)
            QT_sb = consts.tile([P, MC, S], DT)
            KT_sb = consts.tile([P, MC, S], DT)
            V_sb = consts.tile([P, NT, HPC, D + 1], DT)
            OT_sb = consts.tile([P, MC, S], DT)
            nc.vector.memset(V_sb[:, :, :, D : D + 1], 1.0)

            # ---- QKV projections ----
            # K first, then V, then Q -- attention on the first q-chunk can
            # start as soon as Q's first half is done. All evacuations on DVE
            # (tensor_scalar adds bq per-partition) so ACT is free for exp.
            def qk_chunk(w_sb, dst, mc, t4, is_q):
                sl = bass.ts(t4, 512)
                ps = psA.tile(
                    [P, 512], F32, tag="big", name=f"qk{t4}{mc}{int(is_q)}"
                )
                for ec in range(EC):
                    nc.tensor.matmul(
                        ps,
                        lhsT=w_sb[:, ec, mc * P : (mc + 1) * P],
                        rhs=xT_sb[:, ec, sl],
                        start=(ec == 0),
                        stop=(ec == EC - 1),
                    )
                if is_q:
                    nc.vector.tensor_scalar_add(
                        out=dst[:, mc, sl], in0=ps, scalar1=bq_sb[:, mc : mc + 1]
                    )
                else:
                    nc.vector.tensor_copy(out=dst[:, mc, sl], in_=ps)

            def v_tile(t):
                ps = psA.tile([P, EH], F32, tag="big", name=f"v{t}")
                for ec in range(EC):
                    nc.tensor.matmul(
                        ps,
                        lhsT=xT_sb[:, ec, bass.ts(t, P)],
                        rhs=wv_sb[:, ec, :],
                        start=(ec == 0),
                        stop=(ec == EC - 1),
                    )
                nc.vector.tensor_copy(
                    out=V_sb[:, t, :, 0:D],
                    in_=ps.rearrange("p (h d) -> p h d", h=HPC),
                )

            for mc in range(MC):
                qk_chunk(wk_sb, KT_sb, mc, 0, False)
            for t in range(4):
                v_tile(t)
            for t4 in range(2):
                for mc in range(MC):
                    qk_chunk(wq_sb, QT_sb, mc, t4, True)

            # ---- attention + output projection, software pipelined ----
            y_r = y.rearrange("(t p) n -> t p n", p=P)

            def att_unit(hp, iq, Ou, Rs, deferred=()):
                """Scores^T -> exp -> [V|1]^T @ P^T for heads (2hp, 2hp+1) on
                q-chunk iq; evacuates unnormalized O^T + row sums to SBUF.
                `deferred` maps kt -> emit-callback for pipelined fill work."""
                deferred = dict(deferred)
                PT_pairs = [None, None]
                q0 = iq * QH
                O_pair = [
                    psO.tile([D + 1, QH], F32, tag="acc", name=f"O{hp}{iq}a"),
                    psO.tile([D + 1, QH], F32, tag="acc", name=f"O{hp}{iq}b"),
                ]
                for kt in range(NT):
                    if kt in deferred:
                        deferred[kt]()
                    ST_pair = [
                        psA.tile([P, QH], F32, tag="big", name=f"ST{hp}{iq}{kt}a"),
                        psA.tile([P, QH], F32, tag="big", name=f"ST{hp}{iq}{kt}b"),
                    ]
                    # scores^T: row-group packed pair (bases 0 / 64)
                    for qs in range(QH // 512):
                        for i, base in ((0, 0), (1, 64)):
                            nc.tensor.matmul(
                                ST_pair[i][:, bass.ts(qs, 512)],
                                lhsT=KT_sb[base : base + 64, hp, bass.ts(kt, P)],
                                rhs=QT_sb[
                                    base : base + 64,
                                    hp,
                                    q0 + qs * 512 : q0 + (qs + 1) * 512,
                                ],
                                start=True,
                                stop=True,
                            )
                    for i in range(2):
                        h_local = 2 * hp + i
                        PT = work.tile(
                            [P, QH], DT, tag="pt", name=f"PT{hp}{iq}{kt}{i}"
                        )
                        nc.scalar.activation(
                            out=PT, in_=ST_pair[i], func=AF.Exp, scale=SCALE
                        )
                        for qs in range(QH // 512):
                            nc.tensor.matmul(
                                O_pair[i][:, bass.ts(qs, 512)],
                                lhsT=V_sb[:, kt, h_local, :],
                                rhs=PT[:, bass.ts(qs, 512)],
                                start=(kt == 0),
                                stop=(kt == NT - 1),
                            )
                # fast psum evacuation: unnormalized O + approx-recip of sums
                for i in range(2):
                    ou = work.tile([64, QH], F32, tag="ou", name=f"ou{hp}{iq}{i}")
                    nc.vector.tensor_copy(out=ou, in_=O_pair[i][0:D, :])
                    rs = norm.tile([1, QH], F32, tag="rs", bufs=4,
                                   name=f"rs{hp}{iq}{i}")
                    nc.vector.tensor_copy(out=rs, in_=O_pair[i][D : D + 1, :])
                    rc = norm.tile([1, QH], F32, tag="rc", bufs=4,
                                   name=f"rc{hp}{iq}{i}")
                    nc.vector.reciprocal_approx_fast(out=rc, in_=rs)
                    Ou.append(ou)
                    Rs.append(rc)

            def normalize(iq, hp, Ou, Rs):
                """Approx-recip rows -> DMA broadcast -> DVE renorm into OT_sb
                for one (iq, hp) unit; runs concurrently with the next unit."""
                q0 = iq * QH
                rdram = dram.tile([2, QH], F32, tag="rdram", bufs=4,
                                  name=f"rd{iq}{hp}")
                for i in range(2):
                    nc.sync.dma_start(out=rdram[i : i + 1, :], in_=Rs[i])
                bc = norm.tile([64, 2, QH], F32, tag="bc", bufs=4,
                               name=f"bc{iq}{hp}")
                rdram_b = bass.AP(
                    tensor=rdram.tensor,
                    offset=rdram.offset,
                    ap=[[0, 64]] + list(rdram.ap),
                )
                nc.sync.dma_start(out=bc, in_=rdram_b)
                for i in range(2):
                    nc.vector.tensor_mul(
                        out=OT_sb[64 * i : 64 * i + 64, hp, q0 : q0 + QH],
                        in0=Ou[i],
                        in1=bc[:, i, :],
                    )

            def y_tile(t):
                    psY = psA.tile([P, E], F32, tag="big", name=f"psY{t}")
                    for n2 in range(E // 512):
                        for mc in range(MC):
                            nc.tensor.matmul(
                                psY[:, bass.ts(n2, 512)],
                                lhsT=OT_sb[:, mc, bass.ts(t, P)],
                                rhs=wo_sb[:, mc, bass.ts(n2, 512)],
                                start=(mc == 0),
                                stop=(mc == MC - 1),
                            )
                    y_sb = outsb.tile([P, E], F32, tag="ysb", name=f"ysb{t}")
                    nc.vector.tensor_copy(out=y_sb, in_=psY)
                    nc.sync.dma_start(out=y_r[t], in_=y_sb)

            def y_proj(iq):
                for t in range(iq * (NT // NQH), (iq + 1) * (NT // NQH)):
                    y_tile(t)

            # Pipeline: Y(iq) is emitted after att(0, iq+1) so the PE has a
            # full unit of attention work queued before it reaches Y's
            # dependency on the normalize chain (engines run in order).
            def fill(emits):
                d = {}
                for kt, fn in emits:
                    d.setdefault(kt, []).append(fn)
                return {
                    kt: (lambda fns=fns: [f() for f in fns])
                    for kt, fns in d.items()
                }

            for iq in range(NQH):
                for hp in range(MC):
                    Ou, Rs = [], []
                    emits = []
                    if hp == 0 and iq == 0:
                        # remaining K / V chunks, >=2 kts ahead of first use
                        emits += [
                            (0, lambda: qk_chunk(wk_sb, KT_sb, 0, 1, False)),
                            (0, lambda: qk_chunk(wk_sb, KT_sb, 1, 1, False)),
                            (1, lambda: v_tile(4)),
                            (1, lambda: v_tile(5)),
                            (2, lambda: v_tile(6)),
                            (2, lambda: v_tile(7)),
                            (4, lambda: qk_chunk(wk_sb, KT_sb, 0, 2, False)),
                            (4, lambda: qk_chunk(wk_sb, KT_sb, 1, 2, False)),
                            (5, lambda: v_tile(8)),
                            (5, lambda: v_tile(9)),
                            (6, lambda: v_tile(10)),
                            (6, lambda: v_tile(11)),
                            (8, lambda: qk_chunk(wk_sb, KT_sb, 0, 3, False)),
                            (8, lambda: qk_chunk(wk_sb, KT_sb, 1, 3, False)),
                            (9, lambda: v_tile(12)),
                            (9, lambda: v_tile(13)),
                            (10, lambda: v_tile(14)),
                            (10, lambda: v_tile(15)),
                        ]
                    if hp == 1 and iq == 0:
                        # Q chunks for iq1
                        for j, t4 in enumerate((2, 3)):
                            for mc in range(MC):
                                emits.append((
                                    2 * (2 * j + mc),
                                    lambda m=mc, t=t4:
                                    qk_chunk(wq_sb, QT_sb, m, t, True),
                                ))
                    if hp == 0 and iq > 0:
                        base_t = (iq - 1) * (NT // NQH)
                        for j in range(NT // NQH):
                            emits.append((6 + j, lambda t=base_t + j: y_tile(t)))
                    att_unit(hp, iq, Ou, Rs, fill(emits))
                    normalize(iq, hp, Ou, Rs)
            y_proj(NQH - 1)

    nc.compile()
    return nc


_NC_CACHE = {}


def get_nc():
    if "nc" not in _NC_CACHE:
        _NC_CACHE["nc"] = build_nc()
    return _NC_CACHE["nc"]


def make_in_maps(x, Wq, bq, Wk, Wv, Wo):
    xT_by_batch = [
        np.ascontiguousarray(x[b].T).astype(NP_DT) for b in range(B)
    ]
    in_maps = []
    for c in range(NCORES):
        b, hg = divmod(c, NCORES // B)
        hs = slice(hg * EH, (hg + 1) * EH)
        in_maps.append(
            {
                "xT": xT_by_batch[b],
                "wq": np.ascontiguousarray(Wq[:, hs]).astype(NP_DT),
                "wk": np.ascontiguousarray(Wk[:, hs]).astype(NP_DT),
                "wv": np.ascontiguousarray(Wv[:, hs]).astype(NP_DT),
                "wo": np.ascontiguousarray(Wo[hs, :]).astype(NP_DT),
                "bq": np.ascontiguousarray(bq[hs]).astype(np.float32),
            }
        )
    return in_maps


def gather_out(results, bv, Wo, bo):
    bias_row = (
        bv.astype(np.float64) @ Wo.astype(np.float64) + bo.astype(np.float64)
    ).astype(np.float32)
    out = np.empty((B, S, E), np.float32)
    gpb = NCORES // B
    for b in range(B):
        acc = results[gpb * b]["y"].copy()
        for i in range(1, gpb):
            acc += results[gpb * b + i]["y"]
        out[b] = acc + bias_row
    return out


def kernel(x, Wq, bq, Wk, bk, Wv, bv, Wo, bo, **_):
    x = np.asarray(x, np.float32)
    nc = get_nc()
    in_maps = make_in_maps(
        x,
        np.asarray(Wq, np.float32),
        np.asarray(bq, np.float32),
        np.asarray(Wk, np.float32),
        np.asarray(Wv, np.float32),
        np.asarray(Wo, np.float32),
    )
    res = run_bass_kernel_spmd(nc, in_maps, list(range(NCORES)))
    return gather_out(
        res.results, np.asarray(bv, np.float32), np.asarray(Wo, np.float32),
        np.asarray(bo, np.float32)
    )
